# revision 1
# baseline (speedup 1.0000x reference)
"""Multi-head attention (B=4, T=S=2048, H=1024, 16 heads x D=64) on 8 TRN2 cores.

Sharding: 2D mesh of batch(4) x head-group(2). Core c = b*2 + g computes, for
its batch b and its 8 heads (ND slice g*512:(g+1)*512):
  - q/k/v projections (bf16 matmuls, fp32 PSUM accumulate)
  - attention in transposed [S, T] orientation: scoresT = kT.T @ qT chunks,
    exp on ScalarE (1/sqrt(D) folded into the activation scale), softmax
    denominator via a ones-column appended to v in the AV matmul,
    normalization by gpsimd partition-broadcast reciprocal
  - partial output projection out_part = ao @ Wo_g.T  ([T, H], fp32)
Host sums the two head-group partials per batch and adds bo.

ScalarE exp (~33M elements/core) is the roofline (~0.3ms); projection and
output-projection matmuls are emitted through a background queue that
interleaves them between attention s-chunks so TensorE work hides under
the ScalarE stream instead of stalling it.

All matmul inputs bf16: rel err vs fp32 reference ~4e-3. q/k/v biases are
applied in-kernel (zero for this problem, but supported); bo added on host.
"""

from collections import deque

import numpy as np
import ml_dtypes

import concourse.bacc as bacc
import concourse.mybir as mybir
import concourse.tile as tile
from concourse.bass_utils import run_bass_kernel_spmd

B, T, H = 4, 2048, 1024
N_HEADS, D = 16, 64
GROUPS = 2
HEADS_PER_GROUP = N_HEADS // GROUPS          # 8
NDG = HEADS_PER_GROUP * D                    # 512
SCALE = 1.0 / float(D) ** 0.5
N_CORES = 8
TB = 512                                     # attention T-block

bf16 = mybir.dt.bfloat16
f32 = mybir.dt.float32
EXP = mybir.ActivationFunctionType.Exp
MULT = mybir.AluOpType.mult
ADD = mybir.AluOpType.add

_CACHED_NC = None


def _build(repeat=1):
    nc = bacc.Bacc("TRN2", target_bir_lowering=False, debug=False)

    xq_d = nc.dram_tensor("xqT", (H, T), bf16, kind="ExternalInput")
    xv_d = nc.dram_tensor("xvT", (H, T), bf16, kind="ExternalInput")
    wq_d = nc.dram_tensor("wqT", (H, NDG), bf16, kind="ExternalInput")
    wk_d = nc.dram_tensor("wkT", (H, NDG), bf16, kind="ExternalInput")
    wv_d = nc.dram_tensor("wvT", (H, NDG), bf16, kind="ExternalInput")
    wo_d = nc.dram_tensor("woT", (NDG, H), bf16, kind="ExternalInput")
    bq_d = nc.dram_tensor("bq", (NDG,), f32, kind="ExternalInput")
    bk_d = nc.dram_tensor("bk", (NDG,), f32, kind="ExternalInput")
    bv_d = nc.dram_tensor("bv", (NDG,), f32, kind="ExternalInput")
    out_d = nc.dram_tensor("outp", (T, H), f32, kind="ExternalOutput")

    with tile.TileContext(nc) as tc:
        with tc.tile_pool(name="w", bufs=1) as wpool, \
             tc.tile_pool(name="data", bufs=1) as dpool, \
             tc.tile_pool(name="exps", bufs=4) as epool, \
             tc.tile_pool(name="norm", bufs=2) as npool, \
             tc.tile_pool(name="stage", bufs=3) as spool, \
             tc.tile_pool(name="ps_sc", bufs=2, space="PSUM") as ps_sc, \
             tc.tile_pool(name="ps_av", bufs=1, space="PSUM") as ps_av, \
             tc.tile_pool(name="ps_pj", bufs=2, space="PSUM") as ps_pj:

            wq_t = wpool.tile([128, 8, NDG], bf16)
            wk_t = wpool.tile([128, 8, NDG], bf16)
            wv_t = wpool.tile([128, 8, NDG], bf16)
            wo_t = wpool.tile([128, 4, H], bf16)
            bq_t = wpool.tile([128, 4], f32)
            bk_t = wpool.tile([128, 4], f32)
            bv_row = wpool.tile([1, NDG], f32)
            bv_bc = wpool.tile([128, NDG], f32)

            xq_t = dpool.tile([128, 8, T], bf16)
            xv_t = dpool.tile([128, 8, T], bf16)
            qT_t = dpool.tile([128, 4, T], bf16)
            kT_t = dpool.tile([128, 4, T], bf16)
            v_t = dpool.tile([128, 16, HEADS_PER_GROUP, D + 1], bf16)
            ao_t = dpool.tile([128, 4, T], bf16)

            xv_r = xv_d.rearrange("(c p) t -> p c t", p=128)
            xq_r = xq_d.rearrange("(c p) t -> p c t", p=128)
            nc.sync.dma_start(bq_t[:], bq_d.rearrange("(c p) -> p c", p=128))
            nc.sync.dma_start(bk_t[:], bk_d.rearrange("(c p) -> p c", p=128))
            nc.sync.dma_start(bv_row[:], bv_d[None, :])
            nc.sync.dma_start(wk_t[:], wk_d.rearrange("(c p) n -> p c n", p=128))
            nc.scalar.dma_start(wq_t[:], wq_d.rearrange("(c p) n -> p c n", p=128))
            # contiguous per-(h, t-block) chunks: exact (non-overlapping)
            # byte ranges so dependency tracking doesn't serialize falsely
            for h in range(8):
                nc.sync.dma_start(xv_t[:, h, 0:512], xv_r[:, h, 0:512])
                nc.scalar.dma_start(xq_t[:, h, 0:512], xq_r[:, h, 0:512])
            nc.sync.dma_start(wv_t[:], wv_d.rearrange("(c p) n -> p c n", p=128))
            for t4 in range(1, 4):
                for h in range(8):
                    nc.sync.dma_start(xv_t[:, h, t4 * 512:(t4 + 1) * 512],
                                      xv_r[:, h, t4 * 512:(t4 + 1) * 512])
                    nc.scalar.dma_start(xq_t[:, h, t4 * 512:(t4 + 1) * 512],
                                        xq_r[:, h, t4 * 512:(t4 + 1) * 512])
            nc.scalar.dma_start(wo_t[:], wo_d.rearrange("(c p) h -> p c h", p=128))
            nc.gpsimd.partition_broadcast(bv_bc[:], bv_row[0:1, :])
            nc.vector.memset(v_t[:, :, :, D], 1.0)

            # PE warmup while input DMAs stream: spins the HAM clock gate up
            warm = wpool.tile([128, 512], bf16)
            nc.vector.memset(warm[:], 0.0)
            wps = ps_pj.tile([128, 512], f32, tag="pj", name="wps")
            for _ in range(14):
                nc.tensor.matmul(wps[:], warm[:, 0:128], warm[:],
                                 start=True, stop=True)

            # ---- background-emission machinery (PE filler work) ----
            bg = deque()

            def drain(n):
                while n > 0 and bg:
                    try:
                        next(bg[0])
                        n -= 1
                    except StopIteration:
                        bg.popleft()

            def drain_all():
                while bg:
                    drain(64)

            def gen_proj_qk(dst_t, src_t, w_t, b_t, ndc, t4s=range(4)):
                for t4 in t4s:
                    ps = ps_pj.tile([128, 512], f32, tag="pj")
                    for h in range(8):
                        nc.tensor.matmul(
                            ps[:],
                            w_t[:, h, ndc * 128:(ndc + 1) * 128],
                            src_t[:, h, t4 * 512:(t4 + 1) * 512],
                            start=(h == 0), stop=(h == 7),
                        )
                        yield
                    nc.vector.tensor_tensor(
                        dst_t[:, ndc, t4 * 512:(t4 + 1) * 512], ps[:],
                        b_t[:, ndc, None].to_broadcast((128, 512)), ADD)

            def gen_proj_v(t16s=range(16)):
                for t16 in t16s:
                    ps = ps_pj.tile([128, 512], f32, tag="pj")
                    for h in range(8):
                        nc.tensor.matmul(
                            ps[:],
                            xv_t[:, h, t16 * 128:(t16 + 1) * 128],
                            wv_t[:, h, :],
                            start=(h == 0), stop=(h == 7),
                        )
                        yield
                    nc.vector.tensor_tensor(
                        v_t[:, t16, :, 0:D],
                        ps[:].rearrange("p (hh d) -> p hh d", d=D),
                        bv_bc[:].rearrange("p (hh d) -> p hh d", d=D), ADD)

            def gen_oproj(trange):
                for t16 in trange:
                    for hh in range(2):
                        ps = ps_pj.tile([128, 512], f32, tag="pj")
                        for nd in range(4):
                            nc.tensor.matmul(
                                ps[:],
                                ao_t[:, nd, t16 * 128:(t16 + 1) * 128],
                                wo_t[:, nd, hh * 512:(hh + 1) * 512],
                                start=(nd == 0), stop=(nd == 3),
                            )
                            yield
                        st = spool.tile([128, 512], f32, tag="st")
                        if hh == 0:
                            nc.vector.tensor_copy(st[:], ps[:])
                        else:
                            nc.scalar.copy(st[:], ps[:])
                        nc.sync.dma_start(
                            out_d[t16 * 128:(t16 + 1) * 128,
                                  hh * 512:(hh + 1) * 512], st[:])

            def attn_pair(p, pre_chunk=None, pre_av=None, pre_tb=None,
                          post_tb=None, drain_n=3):
                """Heads 2p (partitions 0:64) and 2p+1 (64:128) of chunk p,
                processed together: their score matmuls land in different PE
                row groups and run concurrently; one exp instruction covers
                both heads' [128, 512] score chunks."""
                for tb in range(T // TB):
                    t0 = tb * TB
                    if pre_tb is not None:
                        pre_tb(tb)
                    avAB = ps_av.tile([128, 2 * TB], f32, tag="av",
                                      name="avAB")
                    avA = avAB[:, 0:TB]
                    avB = avAB[:, TB:2 * TB]

                    def av_mms(s, ex):
                        for i, av in ((0, avA), (1, avB)):
                            nc.tensor.matmul(
                                av[0:D + 1, :],
                                v_t[:, s, 2 * p + i, :],
                                ex[:, i * TB:(i + 1) * TB],
                                start=(s == 0), stop=(s == 15),
                            )

                    pending = None
                    for s in range(16):
                        if pre_chunk is not None:
                            pre_chunk(tb, s)
                        sc = ps_sc.tile([128, 2 * TB], f32, tag="sc")
                        for i, off in ((0, 0), (1, 64)):
                            nc.tensor.matmul(
                                sc[:, i * TB:(i + 1) * TB],
                                kT_t[off:off + 64, p, s * 128:(s + 1) * 128],
                                qT_t[off:off + 64, p, t0:t0 + TB],
                                start=True, stop=True,
                            )
                        ex = epool.tile([128, 2 * TB], bf16, tag="exp")
                        nc.scalar.activation(ex[:], sc[:], EXP, scale=SCALE)
                        if pre_av is not None:
                            pre_av(tb, s)
                        if pending is not None:
                            av_mms(*pending)
                        pending = (s, ex)
                        drain(drain_n)
                    av_mms(*pending)
                    for i, av in ((0, avA), (1, avB)):
                        off = 64 * i
                        avs = npool.tile([D + 1, TB], f32, tag="avs")
                        nc.vector.tensor_copy(avs[0:D + 1, :], av[0:D + 1, :])
                        recip = npool.tile([1, TB], f32, tag="recip")
                        nc.vector.reciprocal(recip[:], avs[D:D + 1, :])
                        bc = npool.tile([64, TB], f32, tag="bc")
                        nc.gpsimd.partition_broadcast(bc[:], recip[0:1, :])
                        nc.vector.tensor_tensor(
                            ao_t[off:off + 64, p, t0:t0 + TB],
                            avs[0:D, :], bc[:], MULT)
                    if post_tb is not None:
                        post_tb(tb)

            # ---- emission schedule ----
            def emit_schedule():
              # minimal lead-in: k0 group 0 and q0 group 0 (t cols 0:512)
              for _ in gen_proj_qk(kT_t, xv_t, wk_t, bk_t, 0, [0]):
                pass
              for _ in gen_proj_qk(qT_t, xq_t, wq_t, bq_t, 0, [0]):
                pass

              # pair 0 emits the rest of k0/q0/v inline so attention starts hot:
              # scores(s) needs k0 group s//4; AV(s) needs v group s;
              # t-block tb needs q0 group tb (512-wide blocks)
              def pair0_pre(tb, s):
                if tb == 0 and s in (4, 8, 12):
                    for _ in gen_proj_qk(kT_t, xv_t, wk_t, bk_t, 0, [s // 4]):
                        pass

              def pair0_pre_av(tb, s):
                if tb == 0:
                    for _ in gen_proj_v([s]):
                        pass

              def pair0_pre_tb(tb):
                if tb > 0:
                    for _ in gen_proj_qk(qT_t, xq_t, wq_t, bq_t, 0, [tb]):
                        pass

              bg.append(gen_proj_qk(kT_t, xv_t, wk_t, bk_t, 1))
              bg.append(gen_proj_qk(qT_t, xq_t, wq_t, bq_t, 1))
              attn_pair(0, pre_chunk=pair0_pre, pre_av=pair0_pre_av,
                      pre_tb=pair0_pre_tb)
              drain_all()
              for p in range(1, 4):
                if p < 3:
                    bg.append(gen_proj_qk(kT_t, xv_t, wk_t, bk_t, p + 1))
                    bg.append(gen_proj_qk(qT_t, xq_t, wq_t, bq_t, p + 1))
                last = (p == 3)

                def last_post(tb):
                    # ao rows tb*512..+512 complete for all heads once the
                    # last pair finishes this t-block
                    if tb < 3:
                        bg.append(gen_oproj(range(4 * tb, 4 * tb + 4)))

                attn_pair(p, post_tb=last_post if last else None)
                drain_all()

              for _ in gen_oproj(range(12, 16)):
                pass

            for _rep in range(repeat):
                emit_schedule()

    nc.compile()
    return nc


def kernel(**inputs):
    global _CACHED_NC
    query = np.asarray(inputs["query"], dtype=np.float32)
    value = np.asarray(inputs["value"], dtype=np.float32)
    Wq = np.asarray(inputs["Wq"], dtype=np.float32)
    Wk = np.asarray(inputs["Wk"], dtype=np.float32)
    Wv = np.asarray(inputs["Wv"], dtype=np.float32)
    Wo = np.asarray(inputs["Wo"], dtype=np.float32)
    bq = np.asarray(inputs["bq"], dtype=np.float32)
    bk = np.asarray(inputs["bk"], dtype=np.float32)
    bv = np.asarray(inputs["bv"], dtype=np.float32)
    bo = np.asarray(inputs["bo"], dtype=np.float32)

    if _CACHED_NC is None:
        _CACHED_NC = _build()
    nc = _CACHED_NC

    bf = ml_dtypes.bfloat16
    in_maps = []
    for c in range(N_CORES):
        b, g = c // 2, c % 2
        sl = slice(g * NDG, (g + 1) * NDG)
        in_maps.append({
            "xqT": np.ascontiguousarray(query[b].T).astype(bf),
            "xvT": np.ascontiguousarray(value[b].T).astype(bf),
            "wqT": np.ascontiguousarray(Wq[sl].T).astype(bf),
            "wkT": np.ascontiguousarray(Wk[sl].T).astype(bf),
            "wvT": np.ascontiguousarray(Wv[sl].T).astype(bf),
            "woT": np.ascontiguousarray(Wo[:, sl].T).astype(bf),
            "bq": np.ascontiguousarray(bq[sl]),
            "bk": np.ascontiguousarray(bk[sl]),
            "bv": np.ascontiguousarray(bv[sl]),
        })

    res = run_bass_kernel_spmd(nc, in_maps, core_ids=list(range(N_CORES)))

    out = np.zeros((B, T, H), dtype=np.float32)
    for c in range(N_CORES):
        out[c // 2] += res.results[c]["outp"]
    out += bo
    return out



# revision 10
# speedup vs baseline: 8.4664x; 8.4664x over previous
"""Multi-head attention (B=4, T=S=2048, H=1024, 16 heads x D=64) on 8 TRN2 cores.

Sharding: 2D mesh of batch(4) x head-group(2). Core c = b*2 + g computes, for
its batch b and its 8 heads (ND slice g*512:(g+1)*512):
  - q/k/v projections (bf16 matmuls, fp32 PSUM accumulate)
  - attention in transposed [S, T] orientation: scoresT = kT.T @ qT chunks,
    exp on ScalarE (1/sqrt(D) folded into the activation scale), softmax
    denominator via a ones-column appended to v in the AV matmul,
    normalization by gpsimd partition-broadcast reciprocal
  - partial output projection out_part = ao @ Wo_g.T  ([T, H], fp32)
Host sums the two head-group partials per batch and adds bo.

ScalarE exp (~33M elements/core) is the roofline (~0.3ms); projection and
output-projection matmuls are emitted through a background queue that
interleaves them between attention s-chunks so TensorE work hides under
the ScalarE stream instead of stalling it.

All matmul inputs bf16: rel err vs fp32 reference ~4e-3. q/k/v biases are
applied in-kernel (zero for this problem, but supported); bo added on host.
"""

from collections import deque

import numpy as np
import ml_dtypes

import concourse.bacc as bacc
import concourse.mybir as mybir
import concourse.tile as tile
from concourse.bass_utils import run_bass_kernel_spmd

B, T, H = 4, 2048, 1024
N_HEADS, D = 16, 64
GROUPS = 2
HEADS_PER_GROUP = N_HEADS // GROUPS          # 8
NDG = HEADS_PER_GROUP * D                    # 512
SCALE = 1.0 / float(D) ** 0.5
N_CORES = 8
TB = 512                                     # attention T-block

bf16 = mybir.dt.bfloat16
f32 = mybir.dt.float32
EXP = mybir.ActivationFunctionType.Exp
MULT = mybir.AluOpType.mult
ADD = mybir.AluOpType.add

_CACHED_NC = None


def _build(repeat=1, loop=0):
    """loop=N>0 wraps the schedule in a hardware For_i loop executing the
    full schedule N times (used for steady-state HW timing); loop=0 emits
    the schedule `repeat` times inline (normal path)."""
    nc = bacc.Bacc("TRN2", target_bir_lowering=False, debug=False)

    xq_d = nc.dram_tensor("xqT", (H, T), bf16, kind="ExternalInput")
    xv_d = nc.dram_tensor("xvT", (H, T), bf16, kind="ExternalInput")
    wq_d = nc.dram_tensor("wqT", (H, NDG), bf16, kind="ExternalInput")
    wk_d = nc.dram_tensor("wkT", (H, NDG), bf16, kind="ExternalInput")
    wv_d = nc.dram_tensor("wvT", (H, NDG), bf16, kind="ExternalInput")
    wo_d = nc.dram_tensor("woT", (NDG, H), bf16, kind="ExternalInput")
    bq_d = nc.dram_tensor("bq", (NDG,), f32, kind="ExternalInput")
    bk_d = nc.dram_tensor("bk", (NDG,), f32, kind="ExternalInput")
    bv_d = nc.dram_tensor("bv", (NDG,), f32, kind="ExternalInput")
    out_d = nc.dram_tensor("outp", (T, H), f32, kind="ExternalOutput")

    with tile.TileContext(nc) as tc:
        with tc.tile_pool(name="w", bufs=1) as wpool, \
             tc.tile_pool(name="data", bufs=1) as dpool, \
             tc.tile_pool(name="exps", bufs=4) as epool, \
             tc.tile_pool(name="norm", bufs=2) as npool, \
             tc.tile_pool(name="stage", bufs=3) as spool, \
             tc.tile_pool(name="ps_sc", bufs=2, space="PSUM") as ps_sc, \
             tc.tile_pool(name="ps_av", bufs=1, space="PSUM") as ps_av, \
             tc.tile_pool(name="ps_pj", bufs=2, space="PSUM") as ps_pj:

            wq_t = wpool.tile([128, 8, NDG], bf16)
            wk_t = wpool.tile([128, 8, NDG], bf16)
            wv_t = wpool.tile([128, 8, NDG], bf16)
            wo_t = wpool.tile([128, 4, H], bf16)
            bq_t = wpool.tile([128, 4], f32)
            bk_t = wpool.tile([128, 4], f32)
            bv_row = wpool.tile([1, NDG], f32)
            bv_bc = wpool.tile([128, NDG], f32)

            xq_t = dpool.tile([128, 8, T], bf16)
            xv_t = dpool.tile([128, 8, T], bf16)
            qT_t = dpool.tile([128, 4, T], bf16)
            kT_t = dpool.tile([128, 4, T], bf16)
            v_t = dpool.tile([128, 16, HEADS_PER_GROUP, D + 1], bf16)
            ao_t = dpool.tile([128, 4, T], bf16)

            xv_r = xv_d.rearrange("(c p) t -> p c t", p=128)
            xq_r = xq_d.rearrange("(c p) t -> p c t", p=128)

            # one-time setup (outside the timing loop): ones column for the
            # softmax-denominator trick, PE warmup fodder
            warm = wpool.tile([128, 512], bf16)
            nc.vector.memset(warm[:], 0.0)
            nc.vector.memset(v_t[:, :, :, D], 1.0)
            exd = wpool.tile([1, 16], bf16)

            def emit_inputs():
                """Batched input DMAs: one transfer per 1MB x-chunk instead
                of 8x128KB — the ~2us fixed DMA cost dominated the lead-in.
                Tiny bias loads go on the gpsimd SWDGE queue so they don't
                delay the critical wk/xv_c0 transfers; queues are ordered by
                first-use time (xv chunks feed v-projections early, xq_c1+
                aren't needed until t-block 1)."""
                # ACT table preload off the critical path
                nc.scalar.activation(exd[:], warm[0:1, 0:16], EXP, scale=SCALE)
                nc.gpsimd.dma_start(bq_t[:], bq_d.rearrange("(c p) -> p c", p=128))
                nc.gpsimd.dma_start(bk_t[:], bk_d.rearrange("(c p) -> p c", p=128))
                nc.gpsimd.dma_start(bv_row[:], bv_d[None, :])
                nc.sync.dma_start(wk_t[:], wk_d.rearrange("(c p) n -> p c n", p=128))
                nc.sync.dma_start(xv_t[:, :, 0:512], xv_r[:, :, 0:512])
                nc.scalar.dma_start(wq_t[:], wq_d.rearrange("(c p) n -> p c n", p=128))
                nc.scalar.dma_start(xq_t[:, :, 0:512], xq_r[:, :, 0:512])
                nc.sync.dma_start(wv_t[:], wv_d.rearrange("(c p) n -> p c n", p=128))
                nc.sync.dma_start(xv_t[:, :, 512:1024], xv_r[:, :, 512:1024])
                nc.scalar.dma_start(xv_t[:, :, 1024:1536], xv_r[:, :, 1024:1536])
                nc.sync.dma_start(xv_t[:, :, 1536:2048], xv_r[:, :, 1536:2048])
                for t4 in range(1, 4):
                    nc.scalar.dma_start(xq_t[:, :, t4 * 512:(t4 + 1) * 512],
                                        xq_r[:, :, t4 * 512:(t4 + 1) * 512])
                nc.scalar.dma_start(wo_t[:], wo_d.rearrange("(c p) h -> p c h", p=128))
                nc.gpsimd.partition_broadcast(bv_bc[:], bv_row[0:1, :])

            # PE warmup: spins the HAM clock gate up and bridges the input-DMA
            # latency so the PE doesn't idle into a MID window before k0/q0
            wps = ps_pj.tile([128, 512], f32, tag="pj", name="wps")
            for _ in range(20):
                nc.tensor.matmul(wps[:], warm[:, 0:128], warm[:],
                                 start=True, stop=True)

            # ---- background-emission machinery (PE filler work) ----
            # queue of (key, generator); drain(n) steps n matmuls; force(key)
            # drains until the named generator has fully emitted (hard
            # deadline before emitting a consumer of its output)
            bg = deque()
            bg_done = set()

            def drain(n):
                while n > 0 and bg:
                    try:
                        next(bg[0][1])
                        n -= 1
                    except StopIteration:
                        bg_done.add(bg[0][0])
                        bg.popleft()

            def force(key):
                while bg and key not in bg_done:
                    drain(64)

            def drain_all():
                while bg:
                    drain(64)

            def gen_proj_qk(dst_t, src_t, w_t, b_t, ndc, t4s=range(4)):
                for t4 in t4s:
                    ps = ps_pj.tile([128, 512], f32, tag="pj")
                    for h in range(8):
                        nc.tensor.matmul(
                            ps[:],
                            w_t[:, h, ndc * 128:(ndc + 1) * 128],
                            src_t[:, h, t4 * 512:(t4 + 1) * 512],
                            start=(h == 0), stop=(h == 7),
                        )
                        yield
                    nc.vector.tensor_tensor(
                        dst_t[:, ndc, t4 * 512:(t4 + 1) * 512], ps[:],
                        b_t[:, ndc, None].to_broadcast((128, 512)), ADD)

            def gen_proj_v(t16s=range(16)):
                for t16 in t16s:
                    ps = ps_pj.tile([128, 512], f32, tag="pj")
                    for h in range(8):
                        nc.tensor.matmul(
                            ps[:],
                            xv_t[:, h, t16 * 128:(t16 + 1) * 128],
                            wv_t[:, h, :],
                            start=(h == 0), stop=(h == 7),
                        )
                        yield
                    nc.vector.tensor_tensor(
                        v_t[:, t16, :, 0:D],
                        ps[:].rearrange("p (hh d) -> p hh d", d=D),
                        bv_bc[:].rearrange("p (hh d) -> p hh d", d=D), ADD)

            def gen_oproj(trange):
                for t16 in trange:
                    for hh in range(2):
                        ps = ps_pj.tile([128, 512], f32, tag="pj")
                        for nd in range(4):
                            nc.tensor.matmul(
                                ps[:],
                                ao_t[:, nd, t16 * 128:(t16 + 1) * 128],
                                wo_t[:, nd, hh * 512:(hh + 1) * 512],
                                start=(nd == 0), stop=(nd == 3),
                            )
                            yield
                        st = spool.tile([128, 512], f32, tag="st")
                        nc.vector.tensor_copy(st[:], ps[:])
                        nc.sync.dma_start(
                            out_d[t16 * 128:(t16 + 1) * 128,
                                  hh * 512:(hh + 1) * 512], st[:])

            def attn_pair(p, first=False, post_tb=None, drain_n=3,
                          fine_tail=False):
                """Heads 2p (partitions 0:64) and 2p+1 (64:128) of chunk p,
                processed together: their score matmuls land in different PE
                row groups and run concurrently; one exp instruction covers
                both heads' [128, 512] score chunks.

                first=True: k/q/v tiles for pair 0 are still being produced
                by bg generators — force() their emission right before the
                first consumer so Tile sees the write-before-read order."""
                for tb in range(T // TB):
                    t0 = tb * TB
                    if first and tb > 0:
                        force(("q", 0, tb))
                    avAB = ps_av.tile([128, 2 * TB], f32, tag="av",
                                      name="avAB")
                    avA = avAB[:, 0:TB]
                    avB = avAB[:, TB:2 * TB]

                    def av_mms(s, ex):
                        if first and tb == 0:
                            force(("v", s))
                        for i, av in ((0, avA), (1, avB)):
                            nc.tensor.matmul(
                                av[0:D + 1, :],
                                v_t[:, s, 2 * p + i, :],
                                ex[:, i * TB:(i + 1) * TB],
                                start=(s == 0), stop=(s == 15),
                            )

                    pending = None
                    for s in range(16):
                        if first and tb == 0 and s % 4 == 0 and s > 0:
                            force(("k", 0, s // 4))
                        sc = ps_sc.tile([128, 2 * TB], f32, tag="sc")
                        for i, off in ((0, 0), (1, 64)):
                            nc.tensor.matmul(
                                sc[:, i * TB:(i + 1) * TB],
                                kT_t[off:off + 64, p, s * 128:(s + 1) * 128],
                                qT_t[off:off + 64, p, t0:t0 + TB],
                                start=True, stop=True,
                            )
                        ex = epool.tile([128, 2 * TB], bf16, tag="exp")
                        nc.scalar.activation(ex[:], sc[:], EXP, scale=SCALE)
                        if pending is not None:
                            av_mms(*pending)
                        pending = (s, ex)
                        drain(drain_n)
                    av_mms(*pending)
                    if fine_tail and tb == 3:
                        # last t-block of the last pair: normalize in 128-col
                        # pieces and chain each piece's output projection
                        # immediately, shortening the serial kernel tail
                        drain_all()
                        for q in range(4):
                            cs = slice(t0 + q * 128, t0 + (q + 1) * 128)
                            for i, av in ((0, avA), (1, avB)):
                                off = 64 * i
                                avq = av[:, q * 128:(q + 1) * 128]
                                avs = npool.tile([D + 1, 128], f32, tag="avsf")
                                nc.vector.tensor_copy(avs[:], avq[0:D + 1, :])
                                recip = npool.tile([1, 128], f32, tag="recipf")
                                nc.vector.reciprocal(recip[:], avs[D:D + 1, :])
                                bc = npool.tile([64, 128], f32, tag="bcf")
                                nc.gpsimd.partition_broadcast(bc[:], recip[0:1, :])
                                nc.vector.tensor_tensor(
                                    ao_t[off:off + 64, p, cs],
                                    avs[0:D, :], bc[:], MULT)
                            for _ in gen_oproj([12 + q]):
                                pass
                        continue
                    for i, av in ((0, avA), (1, avB)):
                        off = 64 * i
                        avs = npool.tile([D + 1, TB], f32, tag="avs")
                        nc.vector.tensor_copy(avs[0:D + 1, :], av[0:D + 1, :])
                        recip = npool.tile([1, TB], f32, tag="recip")
                        nc.vector.reciprocal(recip[:], avs[D:D + 1, :])
                        bc = npool.tile([64, TB], f32, tag="bc")
                        nc.gpsimd.partition_broadcast(bc[:], recip[0:1, :])
                        nc.vector.tensor_tensor(
                            ao_t[off:off + 64, p, t0:t0 + TB],
                            avs[0:D, :], bc[:], MULT)
                    if post_tb is not None:
                        post_tb(tb)

            # ---- emission schedule ----
            def emit_schedule():
              emit_inputs()
              # keep the PE busy (and the HAM clock gate open) through the
              # input-DMA wait at the start of each execution: junk matmuls
              # into an sc-pool slot, no data dependencies
              wsc = ps_sc.tile([128, 2 * TB], f32, tag="sc", name="wsc")
              for _ in range(16):
                  nc.tensor.matmul(wsc[:, 0:TB], warm[:, 0:128], warm[:],
                                   start=True, stop=True)
              # minimal lead-in: k0/q0 for s,t cols 0:512; attention starts
              # immediately after, remaining k0/q0/v arrive via the bg queue
              # ordered by first-use time with force() deadlines
              for _ in gen_proj_qk(kT_t, xv_t, wk_t, bk_t, 0, [0]):
                pass
              for _ in gen_proj_qk(qT_t, xq_t, wq_t, bq_t, 0, [0]):
                pass

              def qk(dst, src, w, b, tag, ndc, t4):
                  return ((tag, ndc, t4),
                          gen_proj_qk(dst, src, w, b, ndc, [t4]))

              def kg(ndc, t4):
                  return qk(kT_t, xv_t, wk_t, bk_t, "k", ndc, t4)

              def qg(ndc, t4):
                  return qk(qT_t, xq_t, wq_t, bq_t, "q", ndc, t4)

              def vg(t16):
                  return (("v", t16), gen_proj_v([t16]))

              # need-ordered: v_s needed at AV(s) (chunk s+1); k0_g at chunk
              # 4g; q0_tb at t-block tb (chunk 16*tb)
              bg.extend([
                  vg(0), vg(1), vg(2), vg(3), kg(0, 1), vg(4), vg(5), vg(6),
                  kg(0, 2), vg(7), vg(8), vg(9), vg(10), kg(0, 3), vg(11),
                  vg(12), vg(13), vg(14), vg(15), qg(0, 1), qg(0, 2),
                  qg(0, 3),
                  kg(1, 0), kg(1, 1), kg(1, 2), kg(1, 3),
                  qg(1, 0), qg(1, 1), qg(1, 2), qg(1, 3),
              ])
              attn_pair(0, first=True, drain_n=5)
              drain_all()
              for p in range(1, 4):
                if p < 3:
                    for t4 in range(4):
                        bg.append(kg(p + 1, t4))
                    for t4 in range(4):
                        bg.append(qg(p + 1, t4))
                last = (p == 3)

                def last_post(tb):
                    # ao rows tb*512..+512 complete for all heads once the
                    # last pair finishes this t-block
                    if tb < 3:
                        bg.append((("o", tb), gen_oproj(range(4 * tb, 4 * tb + 4))))

                attn_pair(p, post_tb=last_post if last else None,
                          fine_tail=last)
                drain_all()

            if loop > 0:
                with tc.For_i(0, loop, 1):
                    emit_schedule()
            else:
                for _rep in range(repeat):
                    emit_schedule()

    nc.compile()
    return nc


def kernel(**inputs):
    global _CACHED_NC
    query = np.asarray(inputs["query"], dtype=np.float32)
    value = np.asarray(inputs["value"], dtype=np.float32)
    Wq = np.asarray(inputs["Wq"], dtype=np.float32)
    Wk = np.asarray(inputs["Wk"], dtype=np.float32)
    Wv = np.asarray(inputs["Wv"], dtype=np.float32)
    Wo = np.asarray(inputs["Wo"], dtype=np.float32)
    bq = np.asarray(inputs["bq"], dtype=np.float32)
    bk = np.asarray(inputs["bk"], dtype=np.float32)
    bv = np.asarray(inputs["bv"], dtype=np.float32)
    bo = np.asarray(inputs["bo"], dtype=np.float32)

    if _CACHED_NC is None:
        _CACHED_NC = _build()
    nc = _CACHED_NC

    bf = ml_dtypes.bfloat16
    in_maps = []
    for c in range(N_CORES):
        b, g = c // 2, c % 2
        sl = slice(g * NDG, (g + 1) * NDG)
        in_maps.append({
            "xqT": np.ascontiguousarray(query[b].T).astype(bf),
            "xvT": np.ascontiguousarray(value[b].T).astype(bf),
            "wqT": np.ascontiguousarray(Wq[sl].T).astype(bf),
            "wkT": np.ascontiguousarray(Wk[sl].T).astype(bf),
            "wvT": np.ascontiguousarray(Wv[sl].T).astype(bf),
            "woT": np.ascontiguousarray(Wo[:, sl].T).astype(bf),
            "bq": np.ascontiguousarray(bq[sl]),
            "bk": np.ascontiguousarray(bk[sl]),
            "bv": np.ascontiguousarray(bv[sl]),
        })

    res = run_bass_kernel_spmd(nc, in_maps, core_ids=list(range(N_CORES)))

    out = np.zeros((B, T, H), dtype=np.float32)
    for c in range(N_CORES):
        out[c // 2] += res.results[c]["outp"]
    out += bo
    return out



# revision 35
# speedup vs baseline: 8.8152x; 1.0412x over previous
"""Multi-head attention (B=4, T=S=2048, H=1024, 16 heads x D=64) on 8 TRN2 cores.

Sharding: 2D mesh of batch(4) x head-group(2). Core c = b*2 + g computes, for
its batch b and its 8 heads (ND slice g*512:(g+1)*512):
  - q/k/v projections (bf16 matmuls, fp32 PSUM accumulate)
  - attention in transposed [S, T] orientation: scoresT = kT.T @ qT chunks,
    exp on ScalarE (1/sqrt(D) folded into the activation scale), softmax
    denominator via a ones-column appended to v in the AV matmul,
    normalization by gpsimd partition-broadcast reciprocal
  - partial output projection out_part = ao @ Wo_g.T  ([T, H], fp32)
Host sums the two head-group partials per batch and adds bo.

ScalarE exp (~33M elements/core) is the roofline (~0.3ms); projection and
output-projection matmuls are emitted through a background queue that
interleaves them between attention s-chunks so TensorE work hides under
the ScalarE stream instead of stalling it.

All matmul inputs bf16: rel err vs fp32 reference ~4e-3. q/k/v biases are
applied in-kernel (zero for this problem, but supported); bo added on host.
"""

from collections import deque

import numpy as np
import ml_dtypes

import concourse.bacc as bacc
import concourse.mybir as mybir
import concourse.tile as tile
from concourse.bass_utils import run_bass_kernel_spmd

B, T, H = 4, 2048, 1024
N_HEADS, D = 16, 64
GROUPS = 2
HEADS_PER_GROUP = N_HEADS // GROUPS          # 8
NDG = HEADS_PER_GROUP * D                    # 512
SCALE = 1.0 / float(D) ** 0.5
N_CORES = 8
TB = 512                                     # attention T-block

bf16 = mybir.dt.bfloat16
f32 = mybir.dt.float32
EXP = mybir.ActivationFunctionType.Exp
MULT = mybir.AluOpType.mult
ADD = mybir.AluOpType.add

_CACHED_NC = None


def _build(repeat=1, loop=0, inputs_in_loop=True, mode="full"):
    """loop=N>0 wraps the schedule in a hardware For_i loop executing the
    full schedule N times (used for steady-state HW timing); loop=0 emits
    the schedule `repeat` times inline (normal path). inputs_in_loop=False
    hoists the input DMAs out of the loop body (diagnostic: isolates
    compute-only steady state). mode: "full" | "proj" (projections +
    output projection only) | "attn" (attention core only) — HW probes."""
    nc = bacc.Bacc("TRN2", target_bir_lowering=False, debug=False)

    xq_d = nc.dram_tensor("xqT", (H, T), bf16, kind="ExternalInput")
    xv_d = nc.dram_tensor("xvT", (H, T), bf16, kind="ExternalInput")
    wq_d = nc.dram_tensor("wqT", (H, NDG), bf16, kind="ExternalInput")
    wk_d = nc.dram_tensor("wkT", (H, NDG), bf16, kind="ExternalInput")
    wv_d = nc.dram_tensor("wvT", (H, NDG), bf16, kind="ExternalInput")
    wo_d = nc.dram_tensor("woT", (NDG, H), bf16, kind="ExternalInput")
    bq_d = nc.dram_tensor("bq", (NDG,), f32, kind="ExternalInput")
    bk_d = nc.dram_tensor("bk", (NDG,), f32, kind="ExternalInput")
    bv_d = nc.dram_tensor("bv", (NDG,), f32, kind="ExternalInput")
    out_d = nc.dram_tensor("outp", (T, H), f32, kind="ExternalOutput")

    with tile.TileContext(nc) as tc:
        with tc.tile_pool(name="w", bufs=1) as wpool, \
             tc.tile_pool(name="data", bufs=1) as dpool, \
             tc.tile_pool(name="exps", bufs=4) as epool, \
             tc.tile_pool(name="norm", bufs=2) as npool, \
             tc.tile_pool(name="stage", bufs=3) as spool, \
             tc.tile_pool(name="ps_sc", bufs=2, space="PSUM") as ps_sc, \
             tc.tile_pool(name="ps_av", bufs=1, space="PSUM") as ps_av, \
             tc.tile_pool(name="ps_pj", bufs=2, space="PSUM") as ps_pj:

            wq_t = wpool.tile([128, 8, NDG], bf16)
            wk_t = wpool.tile([128, 8, NDG], bf16)
            wv_t = wpool.tile([128, 8, NDG], bf16)
            wo_t = wpool.tile([128, 4, H], bf16)
            bq_t = wpool.tile([128, 4], f32)
            bk_t = wpool.tile([128, 4], f32)
            bv_row = wpool.tile([1, NDG], f32)
            bv_bc = wpool.tile([128, NDG], f32)

            xq_t = dpool.tile([128, 8, T], bf16)
            xv_t = dpool.tile([128, 8, T], bf16)
            qT_t = dpool.tile([128, 4, T], bf16)
            kT_t = dpool.tile([128, 4, T], bf16)
            v_t = dpool.tile([128, 16, HEADS_PER_GROUP, D + 1], bf16)
            ao_t = dpool.tile([128, 4, T], bf16)

            xv_r = xv_d.rearrange("(c p) t -> p c t", p=128)
            xq_r = xq_d.rearrange("(c p) t -> p c t", p=128)

            # one-time setup (outside the timing loop): ones column for the
            # softmax-denominator trick, PE warmup fodder
            warm = wpool.tile([128, 512], bf16)
            nc.vector.memset(warm[:], 0.0)
            nc.vector.memset(v_t[:, :, :, D], 1.0)
            exd = wpool.tile([1, 16], bf16)
            exst = None
            if mode == "full_nco":
                nc.vector.memset(kT_t[:], 0.0)
                nc.vector.memset(qT_t[:], 0.0)
                nc.vector.memset(v_t[:, :, :, 0:D], 0.0)
                nc.vector.memset(ao_t[:], 0.0)
            if mode.startswith("attn"):
                nc.vector.memset(kT_t[:], 0.0)
                nc.vector.memset(qT_t[:], 0.0)
                nc.vector.memset(v_t[:, :, :, 0:D], 0.0)
                exst = wpool.tile([128, 2 * TB], bf16)
                nc.vector.memset(exst[:], 0.001)
            elif mode == "proj":
                nc.vector.memset(ao_t[:], 0.0)

            def emit_inputs():
                """Batched input DMAs: one transfer per 1MB x-chunk instead
                of 8x128KB — the ~2us fixed DMA cost dominated the lead-in.
                Tiny bias loads go on the gpsimd SWDGE queue so they don't
                delay the critical wk/xv_c0 transfers; queues are ordered by
                first-use time (xv chunks feed v-projections early, xq_c1+
                aren't needed until t-block 1)."""
                # ACT table preload off the critical path
                nc.scalar.activation(exd[:], warm[0:1, 0:16], EXP, scale=SCALE)
                nc.gpsimd.dma_start(bq_t[:], bq_d.rearrange("(c p) -> p c", p=128))
                nc.gpsimd.dma_start(bk_t[:], bk_d.rearrange("(c p) -> p c", p=128))
                nc.gpsimd.dma_start(bv_row[:], bv_d[None, :])
                nc.sync.dma_start(wk_t[:], wk_d.rearrange("(c p) n -> p c n", p=128))
                nc.sync.dma_start(xv_t[:, :, 0:512], xv_r[:, :, 0:512])
                nc.scalar.dma_start(wq_t[:], wq_d.rearrange("(c p) n -> p c n", p=128))
                nc.scalar.dma_start(xq_t[:, :, 0:512], xq_r[:, :, 0:512])
                nc.sync.dma_start(wv_t[:], wv_d.rearrange("(c p) n -> p c n", p=128))
                nc.sync.dma_start(xv_t[:, :, 512:1024], xv_r[:, :, 512:1024])
                nc.scalar.dma_start(xv_t[:, :, 1024:1536], xv_r[:, :, 1024:1536])
                nc.sync.dma_start(xv_t[:, :, 1536:2048], xv_r[:, :, 1536:2048])
                for t4 in range(1, 4):
                    nc.scalar.dma_start(xq_t[:, :, t4 * 512:(t4 + 1) * 512],
                                        xq_r[:, :, t4 * 512:(t4 + 1) * 512])
                nc.scalar.dma_start(wo_t[:], wo_d.rearrange("(c p) h -> p c h", p=128))
                nc.gpsimd.partition_broadcast(bv_bc[:], bv_row[0:1, :])

            # PE warmup: spins the HAM clock gate up and bridges the input-DMA
            # latency so the PE doesn't idle into a MID window before k0/q0
            wps = ps_pj.tile([128, 512], f32, tag="pj", name="wps")
            for _ in range(20):
                nc.tensor.matmul(wps[:], warm[:, 0:128], warm[:],
                                 start=True, stop=True)

            # ---- background-emission machinery (PE filler work) ----
            # queue of (key, generator); drain(n) steps n matmuls; force(key)
            # drains until the named generator has fully emitted (hard
            # deadline before emitting a consumer of its output)
            bg = deque()
            bg_done = set()

            def drain(n):
                while n > 0 and bg:
                    try:
                        next(bg[0][1])
                        n -= 1
                    except StopIteration:
                        bg_done.add(bg[0][0])
                        bg.popleft()

            def force(key):
                while bg and key not in bg_done:
                    drain(64)

            def drain_all():
                while bg:
                    drain(64)

            nco = mode == "full_nco"   # probe: matmuls without copy-outs

            def gen_proj_qk(dst_t, src_t, w_t, b_t, ndc, t4s=range(4)):
                for t4 in t4s:
                    ps = ps_pj.tile([128, 512], f32, tag="pj")
                    for h in range(8):
                        nc.tensor.matmul(
                            ps[:],
                            w_t[:, h, ndc * 128:(ndc + 1) * 128],
                            src_t[:, h, t4 * 512:(t4 + 1) * 512],
                            start=(h == 0), stop=(h == 7),
                        )
                        yield
                    if nco:
                        continue
                    nc.vector.tensor_tensor(
                        dst_t[:, ndc, t4 * 512:(t4 + 1) * 512], ps[:],
                        b_t[:, ndc, None].to_broadcast((128, 512)), ADD)

            def gen_proj_v(t16s=range(16)):
                for t16 in t16s:
                    ps = ps_pj.tile([128, 512], f32, tag="pj")
                    for h in range(8):
                        nc.tensor.matmul(
                            ps[:],
                            xv_t[:, h, t16 * 128:(t16 + 1) * 128],
                            wv_t[:, h, :],
                            start=(h == 0), stop=(h == 7),
                        )
                        yield
                    if nco:
                        continue
                    nc.vector.tensor_tensor(
                        v_t[:, t16, :, 0:D],
                        ps[:].rearrange("p (hh d) -> p hh d", d=D),
                        bv_bc[:].rearrange("p (hh d) -> p hh d", d=D), ADD)

            def gen_oproj(trange):
                for t16 in trange:
                    for hh in range(2):
                        ps = ps_pj.tile([128, 512], f32, tag="pj")
                        for nd in range(4):
                            nc.tensor.matmul(
                                ps[:],
                                ao_t[:, nd, t16 * 128:(t16 + 1) * 128],
                                wo_t[:, nd, hh * 512:(hh + 1) * 512],
                                start=(nd == 0), stop=(nd == 3),
                            )
                            yield
                        if nco:
                            continue
                        st = spool.tile([128, 512], f32, tag="st")
                        nc.vector.tensor_copy(st[:], ps[:])
                        nc.sync.dma_start(
                            out_d[t16 * 128:(t16 + 1) * 128,
                                  hh * 512:(hh + 1) * 512], st[:])

            def attn_pair(p, first=False, post_tb=None, drain_n=3,
                          fine_tail=False, no_exp=False, no_av=False,
                          no_norm=False, norm_upto=-1, drain_skip=0):
                """Heads 2p (partitions 0:64) and 2p+1 (64:128) of chunk p,
                processed together: their score matmuls land in different PE
                row groups and run concurrently; one exp instruction covers
                both heads' [128, 512] score chunks.

                first=True: k/q/v tiles for pair 0 are still being produced
                by bg generators — force() their emission right before the
                first consumer so Tile sees the write-before-read order."""
                for tb in range(T // TB):
                    t0 = tb * TB
                    if first and tb > 0:
                        force(("q", 0, tb))
                    avAB = ps_av.tile([128, 2 * TB], f32, tag="av",
                                      name="avAB")
                    avA = avAB[:, 0:TB]
                    avB = avAB[:, TB:2 * TB]

                    def av_mms(s, ex):
                        if first and tb == 0:
                            force(("v", s))
                        for i, av in ((0, avA), (1, avB)):
                            nc.tensor.matmul(
                                av[0:D + 1, :],
                                v_t[:, s, 2 * p + i, :],
                                ex[:, i * TB:(i + 1) * TB],
                                start=(s == 0), stop=(s == 15),
                            )

                    pending = None
                    for s in range(16):
                        if first and tb == 0 and s % 4 == 0 and s > 0:
                            force(("k", 0, s // 4))
                        # emit the previous chunk's AV matmuls and the bg
                        # filler BEFORE this chunk's score matmuls: the score
                        # stalls on the sc-slot WAR (exp two chunks back), and
                        # PE executes its stream in order — filler placed after
                        # a stalled score would head-of-line block.
                        if not no_av and pending is not None:
                            av_mms(*pending)
                        if s >= drain_skip:
                            drain(drain_n)
                        sc = ps_sc.tile([128, 2 * TB], f32, tag="sc")
                        for i, off in ((0, 0), (1, 64)):
                            nc.tensor.matmul(
                                sc[:, i * TB:(i + 1) * TB],
                                kT_t[off:off + 64, p, s * 128:(s + 1) * 128],
                                qT_t[off:off + 64, p, t0:t0 + TB],
                                start=True, stop=True,
                            )
                        if no_exp:
                            ex = exst  # static tile, probe only
                        else:
                            ex = epool.tile([128, 2 * TB], bf16, tag="exp")
                            nc.scalar.activation(ex[:], sc[:], EXP, scale=SCALE)
                        if not no_av:
                            pending = (s, ex)
                    if not no_av:
                        av_mms(*pending)
                    if no_av or no_norm:
                        continue
                    if norm_upto >= 0:
                        # probe: emit only a prefix of the norm chain
                        for i, av in ((0, avA), (1, avB)):
                            off = 64 * i
                            avs = npool.tile([D + 1, TB], f32, tag="avs")
                            nc.vector.tensor_copy(avs[0:D + 1, :],
                                                  av[0:D + 1, :])
                            if norm_upto >= 1:
                                recip = npool.tile([1, TB], f32, tag="recip")
                                nc.vector.reciprocal(recip[:], avs[D:D + 1, :])
                            if norm_upto >= 2:
                                bc = npool.tile([64, TB], f32, tag="bc")
                                nc.gpsimd.partition_broadcast(bc[:],
                                                              recip[0:1, :])
                        continue
                    if fine_tail and tb == 3:
                        # last t-block of the last pair: normalize in 128-col
                        # pieces and chain each piece's output projection
                        # immediately, shortening the serial kernel tail
                        drain_all()
                        bcs = []
                        for i, av in ((0, avA), (1, avB)):
                            avs = npool.tile([D + 1, TB], f32, tag="avs")
                            nc.vector.tensor_copy(avs[0:D + 1, :],
                                                  av[0:D + 1, :])
                            dt = npool.tile([128, 4], f32, tag="dt")
                            nc.scalar.dma_start(dt[:, :], avs[D:D + 1, :])
                            rt = npool.tile([128, 4], f32, tag="rt")
                            nc.vector.reciprocal(rt[:], dt[:])
                            recip = npool.tile([1, TB], f32, tag="recip")
                            nc.scalar.dma_start(recip[0:1, :], rt[:, :])
                            bc = npool.tile([64, TB], f32, tag="bc")
                            nc.gpsimd.partition_broadcast(bc[:], recip[0:1, :])
                            bcs.append((64 * i, avs, bc))
                        # per-128-col multiply + immediate output projection:
                        # shortens the serial tail after the last AV matmul
                        for q in range(4):
                            cq = slice(q * 128, (q + 1) * 128)
                            cs = slice(t0 + q * 128, t0 + (q + 1) * 128)
                            for off, avs, bc in bcs:
                                nc.vector.tensor_tensor(
                                    ao_t[off:off + 64, p, cs],
                                    avs[0:D, cq], bc[:, cq], MULT)
                            for _ in gen_oproj([12 + q]):
                                pass
                        continue
                    for i, av in ((0, avA), (1, avB)):
                        off = 64 * i
                        avs = npool.tile([D + 1, TB], f32, tag="avs")
                        nc.vector.tensor_copy(avs[0:D + 1, :], av[0:D + 1, :])
                        # reciprocal of the 512 softmax denominators: the DVE
                        # iterative divide is serial along the free dim, so a
                        # [1,512] recip costs ~1.5us. DMA-scatter the row to
                        # [128,4] (partition-parallel recip, ~30ns), gather
                        # back. The tiny DMAs ride the otherwise-idle scalar
                        # HWDGE queue.
                        dt = npool.tile([128, 4], f32, tag="dt")
                        nc.scalar.dma_start(dt[:, :], avs[D:D + 1, :])
                        rt = npool.tile([128, 4], f32, tag="rt")
                        nc.vector.reciprocal(rt[:], dt[:])
                        recip = npool.tile([1, TB], f32, tag="recip")
                        nc.scalar.dma_start(recip[0:1, :], rt[:, :])
                        bc = npool.tile([64, TB], f32, tag="bc")
                        nc.gpsimd.partition_broadcast(bc[:], recip[0:1, :])
                        nc.vector.tensor_tensor(
                            ao_t[off:off + 64, p, t0:t0 + TB],
                            avs[0:D, :], bc[:], MULT)
                    if post_tb is not None:
                        post_tb(tb)

            def emit_schedule_proj():
                # probe: q/k/v projections + output projection, no attention
                emit_inputs()
                for ndc in range(4):
                    for _ in gen_proj_qk(kT_t, xv_t, wk_t, bk_t, ndc):
                        pass
                    for _ in gen_proj_qk(qT_t, xq_t, wq_t, bq_t, ndc):
                        pass
                for _ in gen_proj_v():
                    pass
                for _ in gen_oproj(range(16)):
                    pass

            def emit_schedule_attn():
                # probe: attention core only (qT/kT/v memset once outside)
                kw = {}
                if mode == "attn_noexp":
                    kw = dict(no_exp=True)
                elif mode == "attn_noav":
                    kw = dict(no_av=True)
                elif mode == "attn_nonorm":
                    kw = dict(no_norm=True)
                elif mode.startswith("attn_n"):
                    kw = dict(norm_upto=int(mode[-1]))
                for p in range(4):
                    attn_pair(p, **kw)

            # ---- emission schedule ----
            def emit_schedule():
              if mode == "proj":
                  emit_schedule_proj()
                  return
              if mode.startswith("attn"):
                  emit_schedule_attn()
                  return
              if inputs_in_loop:
                  emit_inputs()
              # keep the PE busy (and the HAM clock gate open) through the
              # input-DMA wait at the start of each execution: junk matmuls
              # into an sc-pool slot, no data dependencies
              wsc = ps_sc.tile([128, 2 * TB], f32, tag="sc", name="wsc")
              for _ in range(16):
                  nc.tensor.matmul(wsc[:, 0:TB], warm[:, 0:128], warm[:],
                                   start=True, stop=True)
              # minimal lead-in: k0/q0 for s,t cols 0:512; attention starts
              # immediately after, remaining k0/q0/v arrive via the bg queue
              # ordered by first-use time with force() deadlines
              for _ in gen_proj_qk(kT_t, xv_t, wk_t, bk_t, 0, [0]):
                pass
              for _ in gen_proj_qk(qT_t, xq_t, wq_t, bq_t, 0, [0]):
                pass

              def qk(dst, src, w, b, tag, ndc, t4):
                  return ((tag, ndc, t4),
                          gen_proj_qk(dst, src, w, b, ndc, [t4]))

              def kg(ndc, t4):
                  return qk(kT_t, xv_t, wk_t, bk_t, "k", ndc, t4)

              def qg(ndc, t4):
                  return qk(qT_t, xq_t, wq_t, bq_t, "q", ndc, t4)

              def vg(t16):
                  return (("v", t16), gen_proj_v([t16]))

              # need-ordered: v_s needed at AV(s) (chunk s+1); k0_g at chunk
              # 4g; q0_tb at t-block tb (chunk 16*tb)
              bg.extend([
                  vg(0), vg(1), vg(2), vg(3), kg(0, 1), vg(4), vg(5), vg(6),
                  kg(0, 2), vg(7), vg(8), vg(9), vg(10), kg(0, 3), vg(11),
                  vg(12), vg(13), vg(14), vg(15), qg(0, 1), qg(0, 2),
                  qg(0, 3),
                  kg(1, 0), kg(1, 1), kg(1, 2), kg(1, 3),
                  qg(1, 0), qg(1, 1), qg(1, 2), qg(1, 3),
              ])
              attn_pair(0, first=True, drain_n=5)
              drain_all()
              for p in range(1, 4):
                if p < 3:
                    for t4 in range(4):
                        bg.append(kg(p + 1, t4))
                    for t4 in range(4):
                        bg.append(qg(p + 1, t4))
                last = (p == 3)

                def last_post(tb):
                    # ao rows tb*512..+512 complete for all heads once the
                    # last pair finishes this t-block
                    if tb < 3:
                        bg.append((("o", tb), gen_oproj(range(4 * tb, 4 * tb + 4))))

                attn_pair(p, post_tb=last_post if last else None,
                          fine_tail=last, drain_skip=2 if last else 0)
                drain_all()

            if loop > 0:
                if not inputs_in_loop:
                    emit_inputs()
                with tc.For_i(0, loop, 1):
                    emit_schedule()
            else:
                for _rep in range(repeat):
                    emit_schedule()

    nc.compile()
    return nc


def kernel(**inputs):
    global _CACHED_NC
    query = np.asarray(inputs["query"], dtype=np.float32)
    value = np.asarray(inputs["value"], dtype=np.float32)
    Wq = np.asarray(inputs["Wq"], dtype=np.float32)
    Wk = np.asarray(inputs["Wk"], dtype=np.float32)
    Wv = np.asarray(inputs["Wv"], dtype=np.float32)
    Wo = np.asarray(inputs["Wo"], dtype=np.float32)
    bq = np.asarray(inputs["bq"], dtype=np.float32)
    bk = np.asarray(inputs["bk"], dtype=np.float32)
    bv = np.asarray(inputs["bv"], dtype=np.float32)
    bo = np.asarray(inputs["bo"], dtype=np.float32)

    if _CACHED_NC is None:
        _CACHED_NC = _build()
    nc = _CACHED_NC

    bf = ml_dtypes.bfloat16
    in_maps = []
    for c in range(N_CORES):
        b, g = c // 2, c % 2
        sl = slice(g * NDG, (g + 1) * NDG)
        in_maps.append({
            "xqT": np.ascontiguousarray(query[b].T).astype(bf),
            "xvT": np.ascontiguousarray(value[b].T).astype(bf),
            "wqT": np.ascontiguousarray(Wq[sl].T).astype(bf),
            "wkT": np.ascontiguousarray(Wk[sl].T).astype(bf),
            "wvT": np.ascontiguousarray(Wv[sl].T).astype(bf),
            "woT": np.ascontiguousarray(Wo[:, sl].T).astype(bf),
            "bq": np.ascontiguousarray(bq[sl]),
            "bk": np.ascontiguousarray(bk[sl]),
            "bv": np.ascontiguousarray(bv[sl]),
        })

    res = run_bass_kernel_spmd(nc, in_maps, core_ids=list(range(N_CORES)))

    out = np.zeros((B, T, H), dtype=np.float32)
    for c in range(N_CORES):
        out[c // 2] += res.results[c]["outp"]
    out += bo
    return out

